# revision 27
# baseline (speedup 1.0000x reference)
"""Trainium2 Bass kernel: fused multi-head attention (dense transformer block).

Reference computation (per batch element b of 8, one NeuronCore each):
    qkv = x @ w_qkv.T                  # [1024, 2304]
    q, k, v = split(qkv); reshape to 12 heads x 64 dims
    s = q @ k.T (unscaled); p = softmax(s); o = p @ v
    out = concat_heads(o) @ w_fc.T + b_fc

Kernel layout strategy (all per-core):
  - All operands arrive PRE-TRANSPOSED from the host wrapper (x.T, w_qkv.T,
    w_fc.T, broadcast bias): layout prep is the shard-side job of kernel(),
    so the NeuronCore spends zero cycles on PE transposes.
  - Everything runs in "transposed" layout: q_T/k_T are [head_dim, seq] so the
    TensorEngine can contract over head_dim directly; scores are computed as
    S_T[k, q] (keys on partitions) so exp needs no transpose and P_T feeds the
    P@V matmul as the moving operand.
  - The two heads of a pair occupy PE row-tiles (0-63 / 64-127): their score
    matmuls run concurrently in the 64x128-tiled array.
  - Softmax skips max-subtraction (scores are bounded ~|70| < 88 overflow
    limit) and gets the denominator for free by appending a ones-column to V
    (M=65 output rows; row 64 = sum_k P).
  - Normalization: denominator row -> DRAM-bounce reshape to [128,8] ->
    wide reciprocal -> DMA partition-broadcast -> one vector multiply.
  - The fc output is produced in natural [seq, dim] layout by using ao_T as
    the stationary operand, so no final transpose is needed.
  - Precision: qkv + scores run in float32r (TF32-like, ~1.6e-4 rel err,
    full PE speed); P, V, ao, w_fc in bf16. End-to-end ~3e-3 max rel err.
  - The whole kernel is software-pipelined per head pair: pair p's qkv is
    prefetched one pair ahead, S(p) is chunk-interleaved with PV(p-1) and
    bg work (qkv matmuls for later pairs, v matmuls), so ScalarE's exp
    stream (the ~100us co-bottleneck) starts early and stays fed.
"""

import numpy as np
import ml_dtypes
import concourse.bacc as bacc
import concourse.mybir as mybir
import concourse.tile as tile
from concourse.bass_utils import run_bass_kernel_spmd

SEQ = 1024
DIM = 768
H = 12
DH = 64
E = 3 * DIM  # 2304
NT = SEQ // 128  # 8  seq chunks
DT = DIM // 128  # 6  dim chunks
VA = H * (DH + 1)  # 780: v with ones column per head

f32 = mybir.dt.float32
f32r = mybir.dt.float32r
bf16 = mybir.dt.bfloat16
EXP = mybir.ActivationFunctionType.Exp


def build():
    nc = bacc.Bacc("TRN2", target_bir_lowering=False, debug=False)
    xt_d = nc.dram_tensor("x_t", [DIM, SEQ], f32r, kind="ExternalInput")
    wqkvt_d = nc.dram_tensor("wqkv_t", [DIM, E], f32r, kind="ExternalInput")
    wfct_d = nc.dram_tensor("wfc_t", [DIM, DIM], bf16, kind="ExternalInput")
    biasbc_d = nc.dram_tensor("bias_bc", [128, DIM], f32,
                              kind="ExternalInput")
    out_d = nc.dram_tensor("out", [SEQ, DIM], f32, kind="ExternalOutput")

    with tile.TileContext(nc) as tc:
        with (
            tc.tile_pool(name="const", bufs=1) as constp,
            tc.tile_pool(name="persist", bufs=1) as persist,
            tc.tile_pool(name="work", bufs=1) as work,
            tc.tile_pool(name="dsc", bufs=1, space="DRAM") as dscp,
            tc.tile_pool(name="ps", bufs=1, space="PSUM") as psp,
        ):
            # persistent tensors
            bias_bc = constp.tile([128, DIM], f32, tag="bbc")
            va = [persist.tile([128, VA], bf16, tag=f"va{nt}", name=f"va{nt}")
                  for nt in range(NT)]
            aoT = [persist.tile([128, SEQ], bf16, tag=f"ao{j}", name=f"aoT{j}")
                   for j in range(DT)]
            wfcT = [persist.tile([128, DIM], bf16, tag=f"wfcT{j}",
                                 name=f"wfcT{j}") for j in range(DT)]
            xT = [persist.tile([128, SEQ], f32r, tag=f"xT{j}", name=f"xT{j}")
                  for j in range(DT)]
            wvT = [persist.tile([128, DIM], f32r, tag=f"wvT{j}",
                                name=f"wvT{j}") for j in range(DT)]

            # ---- persistent loads: everything is pre-transposed in DRAM ----
            def load_persistent():
                for j in range(DT):
                    nc.sync.dma_start(xT[j][:],
                                      xt_d.ap()[j * 128:(j + 1) * 128, :])
                for j in range(DT):
                    nc.sync.dma_start(
                        wvT[j][:],
                        wqkvt_d.ap()[j * 128:(j + 1) * 128,
                                     12 * 128:18 * 128])
                for j in range(DT):
                    nc.sync.dma_start(wfcT[j][:],
                                      wfct_d.ap()[j * 128:(j + 1) * 128, :])
                nc.sync.dma_start(bias_bc[:], biasbc_d.ap())

            # ---- v matmuls as 16 self-contained parts (one seq-chunk
            # half each), slotted into the early pairs' chunk streams ----
            def v_parts():
                def vp(nt, h2):
                    lo, hi = (0, 512) if h2 == 0 else (512, 768)
                    def go():
                        psv = psp.tile([128, hi - lo], f32, tag="mm", bufs=2,
                                       name="psv")
                        for j in range(DT):
                            nc.tensor.matmul(psv[:],
                                             xT[j][:, nt * 128:(nt + 1) * 128],
                                             wvT[j][:, lo:hi],
                                             start=(j == 0),
                                             stop=(j == DT - 1))
                        va3 = va[nt][:].rearrange("p (h c) -> p h c", c=DH + 1)
                        nc.vector.tensor_copy(
                            va3[:, lo // DH:hi // DH, 0:DH],
                            psv[:].rearrange("p (h c) -> p h c", c=DH))
                        if h2 == 1:
                            nc.gpsimd.memset(va3[:, :, DH:DH + 1], 1.0)
                    return go
                return [vp(nt, h2) for nt in range(NT) for h2 in range(2)]

            def wt_qkv_parts(p, tiles, split_load=False):
                """Pair p's q_T/k_T: w chunks stream in pre-transposed via
                DMA; 4 matmul parts produce qt/kt in `tiles`."""
                wq_t = [work.tile([128, 256], f32r, tag=f"wq{j}", bufs=2,
                                  name=f"wq{j}_{p}") for j in range(DT)]

                def load():
                    for j in range(DT):
                        for ci, et in enumerate((p, 6 + p)):
                            nc.sync.dma_start(
                                wq_t[j][:, ci * 128:(ci + 1) * 128],
                                wqkvt_d.ap()[j * 128:(j + 1) * 128,
                                             et * 128:(et + 1) * 128])

                def qkmm(ci, half, h2):
                    ps = psp.tile([128, 512], f32, tag="mm", bufs=2,
                                  name="ps")
                    for j in range(DT):
                        nc.tensor.matmul(
                            ps[:],
                            wq_t[j][:, ci * 128:(ci + 1) * 128],
                            xT[j][:, h2 * 512:(h2 + 1) * 512],
                            start=(j == 0), stop=(j == DT - 1))
                    if h2 == 0:
                        t = work.tile([128, SEQ], f32r,
                                      tag=f"qk_{half}{p % 3}", bufs=1,
                                      name=f"qk{half}{p}")
                        tiles[half] = t
                    nc.vector.tensor_copy(
                        tiles[half][:, h2 * 512:(h2 + 1) * 512], ps[:])

                mm_parts = [
                    lambda: qkmm(0, "q", 0), lambda: qkmm(0, "q", 1),
                    lambda: qkmm(1, "k", 0), lambda: qkmm(1, "k", 1)]
                if split_load:
                    return load, mm_parts
                return [load] + mm_parts

            def run_parts(parts):
                for f in parts:
                    f()

            def normalize(p, xi, st):
                """recip of denominator row via DRAM-bounce, broadcast, mul."""
                dsc2 = dscp.tile([1, SEQ], f32, tag="dsc2", bufs=2,
                                 name="dsc2")
                dsc1 = dscp.tile([1, SEQ], f32, tag="dsc1", bufs=2,
                                 name="dsc1")
                nc.sync.dma_start(dsc1[:], st[DH:DH + 1, :])
                den8 = work.tile([128, 8], f32, tag="den8", bufs=2,
                                 name="den8")
                nc.sync.dma_start(
                    den8[:], dsc1[:].rearrange("a (p c) -> (a p) c", c=8))
                recip8 = work.tile([128, 8], f32, tag="recip8", bufs=2,
                                   name="recip8")
                nc.vector.reciprocal(recip8[:], den8[:])
                nc.sync.dma_start(
                    dsc2[:].rearrange("a (p c) -> (a p) c", c=8), recip8[:])
                bc_sb = work.tile([64, SEQ], f32, tag="bc", bufs=2,
                                  name="bc_sb")
                nc.sync.dma_start(bc_sb[:], dsc2[:].broadcast_to([64, SEQ]))
                nc.vector.tensor_mul(
                    aoT[p][xi * 64:(xi + 1) * 64, :], st[0:DH, :], bc_sb[:])

            def drain_po(p, xi, po):
                """Stage [65, SEQ] out of PSUM in one copy, then normalize.
                Copy runs on DVE, keeping ScalarE free for the exp stream."""
                st = work.tile([DH + 1, SEQ], f32, tag="stage", bufs=2,
                               name="st")
                nc.vector.tensor_copy(st[:], po[:])
                normalize(p, xi, st)

            def pair_step(p, qk, PT_prev, bg=()):
                """S(p) chunk-interleaved with PV(p-1) + background parts;
                returns PT(p)."""
                qt, kt = qk["q"], qk["k"]
                L = len(bg)
                PT = {}
                po = {}
                if PT_prev is not None:
                    for xi in range(2):
                        po[xi] = psp.tile([DH + 1, SEQ], f32, tag=f"o{xi}",
                                          bufs=1, name=f"po{xi}")
                for c in range(NT):
                    # bg parts run first: their PSUM allocations then reuse
                    # buffers whose exp (c-1) has already drained, instead of
                    # stalling the PE behind an in-flight exp.
                    for i in range(L * c // NT, L * (c + 1) // NT):
                        bg[i]()
                    if PT_prev is not None:
                        for xi in range(2):
                            hX = 2 * (p - 1) + xi
                            va_h = va[c][:, hX * (DH + 1):(hX + 1) * (DH + 1)]
                            for h2 in range(2):
                                nc.tensor.matmul(
                                    po[xi][:, h2 * 512:(h2 + 1) * 512],
                                    va_h,
                                    PT_prev[(xi, c)][:, h2 * 512:
                                                     (h2 + 1) * 512],
                                    start=(c == 0), stop=(c == NT - 1))
                    # All 4 S matmuls first (the two xi target different
                    # 64-row PE tiles, so B's mms overlap A's), exps after —
                    # no ACT op ever sits between PE instructions.
                    ps_s = {}
                    for xi in range(2):
                        ro = xi * 64
                        ps = psp.tile([128, SEQ], f32, tag="mm", bufs=2,
                                      name="ps_s")
                        for h2 in range(2):
                            nc.tensor.matmul(
                                ps[:, h2 * 512:(h2 + 1) * 512],
                                kt[ro:ro + 64, c * 128:(c + 1) * 128],
                                qt[ro:ro + 64, h2 * 512:(h2 + 1) * 512],
                                start=True, stop=True)
                        ps_s[xi] = ps
                    for xi in range(2):
                        pt = work.tile([128, SEQ], bf16, tag=f"pt{xi}_{c}",
                                       bufs=1, name="pt")
                        nc.scalar.activation(pt[:], ps_s[xi][:], EXP)
                        PT[(xi, c)] = pt
                if PT_prev is not None:
                    for xi in range(2):
                        drain_po(p - 1, xi, po[xi])
                return PT

            def pv_only(p, PT_prev):
                for xi in range(2):
                    po = psp.tile([DH + 1, SEQ], f32, tag=f"o{xi}", bufs=1,
                                  name=f"po{xi}")
                    for c in range(NT):
                        hX = 2 * p + xi
                        va_h = va[c][:, hX * (DH + 1):(hX + 1) * (DH + 1)]
                        for h2 in range(2):
                            nc.tensor.matmul(
                                po[:, h2 * 512:(h2 + 1) * 512],
                                va_h,
                                PT_prev[(xi, c)][:, h2 * 512:(h2 + 1) * 512],
                                start=(c == 0), stop=(c == NT - 1))
                    drain_po(p, xi, po)

            # ---- preamble: pure DMA, then straight into pair 0 ----
            qk_tiles = {p: {} for p in range(6)}
            w0_load, w0_parts = wt_qkv_parts(0, qk_tiles[0], split_load=True)
            load_persistent()
            w0_load()
            run_parts(w0_parts)
            w1 = wt_qkv_parts(1, qk_tiles[1])
            vps = v_parts()

            def merge(a, b):
                out, ia, ib = [], 0, 0
                while ia < len(a) or ib < len(b):
                    if ia * len(b) <= ib * len(a) and ia < len(a):
                        out.append(a[ia]); ia += 1
                    elif ib < len(b):
                        out.append(b[ib]); ib += 1
                    else:
                        out.append(a[ia]); ia += 1
                return out

            bg_sched = {
                0: merge(w1, vps[:12]),
                1: vps[12:] + wt_qkv_parts(2, qk_tiles[2]),
                2: wt_qkv_parts(3, qk_tiles[3]),
                3: wt_qkv_parts(4, qk_tiles[4]),
                4: wt_qkv_parts(5, qk_tiles[5]),
                5: [],
            }
            PT_cur = None
            for p in range(6):
                PT_cur = pair_step(p, qk_tiles[p], PT_cur, bg_sched[p])
            pv_only(5, PT_cur)

            # ---- fc + bias, natural layout ----
            for nt in range(NT):
                psy = psp.tile([128, DIM], f32, tag="mm", bufs=2, name="psy")
                for j in range(DT):
                    nc.tensor.matmul(psy[:, 0:512],
                                     aoT[j][:, nt * 128:(nt + 1) * 128],
                                     wfcT[j][:, 0:512],
                                     start=(j == 0), stop=(j == DT - 1))
                    nc.tensor.matmul(psy[:, 512:768],
                                     aoT[j][:, nt * 128:(nt + 1) * 128],
                                     wfcT[j][:, 512:768],
                                     start=(j == 0), stop=(j == DT - 1))
                y = work.tile([128, DIM], f32, tag="y_sb", bufs=2, name="y")
                nc.vector.tensor_add(y[:], psy[:], bias_bc[:])
                nc.sync.dma_start(out_d.ap()[nt * 128:(nt + 1) * 128, :], y[:])

    nc.compile()
    return nc


_NC = None
LAST_RESULTS = None  # BassKernelResults of the most recent run (for profiling)


def kernel(**inputs) -> np.ndarray:
    global _NC, LAST_RESULTS
    x = np.asarray(inputs["x"], dtype=np.float32)
    w_qkv = np.asarray(inputs["w_qkv"], dtype=np.float32)
    w_fc = np.asarray(inputs["w_fc"], dtype=np.float32)
    b_fc = np.asarray(inputs["b_fc"], dtype=np.float32).reshape(1, DIM)

    # Host-side shard/layout prep: pre-transpose operands so the NeuronCore
    # never runs a PE transpose.
    wqkv_t = np.ascontiguousarray(w_qkv.T)                      # [768, 2304]
    wfc_t = np.ascontiguousarray(w_fc.T).astype(ml_dtypes.bfloat16)
    bias_bc = np.ascontiguousarray(
        np.broadcast_to(b_fc, (128, DIM)).astype(np.float32))

    if _NC is None:
        _NC = build()
    nc = _NC

    in_maps = [
        {"x_t": np.ascontiguousarray(x[b].T), "wqkv_t": wqkv_t,
         "wfc_t": wfc_t, "bias_bc": bias_bc}
        for b in range(8)
    ]
    res = run_bass_kernel_spmd(nc, in_maps, core_ids=list(range(8)))
    LAST_RESULTS = res
    out = np.stack([r["out"] for r in res.results], axis=0)
    return out.astype(np.float32)


if __name__ == "__main__":
    rng = np.random.default_rng(0)
    ins = {
        "x": rng.standard_normal((8, SEQ, DIM), dtype=np.float32),
        "w_qkv": (rng.standard_normal((E, DIM), dtype=np.float32) * DIM ** -0.5),
        "w_fc": (rng.standard_normal((DIM, DIM), dtype=np.float32) * DIM ** -0.5),
        "b_fc": (rng.standard_normal((DIM,), dtype=np.float32) * 0.02),
    }
    out = kernel(**ins)
    print("out", out.shape, out.dtype)


# revision 33
# speedup vs baseline: 1.2093x; 1.2093x over previous
"""Trainium2 Bass kernel: fused multi-head attention (dense transformer block).

Reference computation (per batch element b of 8, one NeuronCore each):
    qkv = x @ w_qkv.T                  # [1024, 2304]
    q, k, v = split(qkv); reshape to 12 heads x 64 dims
    s = q @ k.T (unscaled); p = softmax(s); o = p @ v
    out = concat_heads(o) @ w_fc.T + b_fc

Kernel layout strategy (all per-core):
  - All operands arrive PRE-TRANSPOSED from the host wrapper (x.T, w_qkv.T,
    w_fc.T, broadcast bias): layout prep is the shard-side job of kernel(),
    so the NeuronCore spends zero cycles on PE transposes.
  - Everything runs in "transposed" layout: q_T/k_T are [head_dim, seq] so the
    TensorEngine can contract over head_dim directly; scores are computed as
    S_T[k, q] (keys on partitions) so exp needs no transpose and P_T feeds the
    P@V matmul as the moving operand.
  - The two heads of a pair occupy PE row-tiles (0-63 / 64-127): their score
    matmuls run concurrently in the 64x128-tiled array.
  - Softmax skips max-subtraction (scores are bounded ~|70| < 88 overflow
    limit) and gets the denominator for free by appending a ones-column to V
    (M=65 output rows; row 64 = sum_k P).
  - Normalization: denominator row -> DRAM-bounce reshape to [128,8] ->
    wide reciprocal -> DMA partition-broadcast -> one vector multiply.
  - The fc output is produced in natural [seq, dim] layout by using ao_T as
    the stationary operand, so no final transpose is needed.
  - Precision: qkv + scores run in float32r (TF32-like, ~1.6e-4 rel err,
    full PE speed); P, V, ao, w_fc in bf16. End-to-end ~3e-3 max rel err.
  - The whole kernel is software-pipelined per head pair: pair p's qkv is
    prefetched one pair ahead, S(p) is chunk-interleaved with PV(p-1) and
    bg work (qkv matmuls for later pairs, v matmuls), so ScalarE's exp
    stream (the ~100us co-bottleneck) starts early and stays fed.
"""

import numpy as np
import ml_dtypes
import concourse.bacc as bacc
import concourse.mybir as mybir
import concourse.tile as tile
from concourse.bass_utils import run_bass_kernel_spmd

SEQ = 1024
DIM = 768
H = 12
DH = 64
E = 3 * DIM  # 2304
NT = SEQ // 128  # 8  seq chunks
DT = DIM // 128  # 6  dim chunks
VA = H * (DH + 1)  # 780: v with ones column per head

f32 = mybir.dt.float32
f32r = mybir.dt.float32r
bf16 = mybir.dt.bfloat16
EXP = mybir.ActivationFunctionType.Exp


def build():
    nc = bacc.Bacc("TRN2", target_bir_lowering=False, debug=False)
    xt_d = nc.dram_tensor("x_t", [DIM, SEQ], f32r, kind="ExternalInput")
    wqkvt_d = nc.dram_tensor("wqkv_t", [DIM, E], f32r, kind="ExternalInput")
    wfct_d = nc.dram_tensor("wfc_t", [DIM, DIM], bf16, kind="ExternalInput")
    bias6_d = nc.dram_tensor("bias6", [128, DT], f32, kind="ExternalInput")
    # out is produced TRANSPOSED [dim, seq]; the host wrapper un-transposes.
    out_d = nc.dram_tensor("out", [DIM, SEQ], f32, kind="ExternalOutput")

    with tile.TileContext(nc) as tc:
        with (
            tc.tile_pool(name="const", bufs=1) as constp,
            tc.tile_pool(name="persist", bufs=1) as persist,
            tc.tile_pool(name="work", bufs=1) as work,
            tc.tile_pool(name="dsc", bufs=1, space="DRAM") as dscp,
            tc.tile_pool(name="ps", bufs=1, space="PSUM") as psp,
        ):
            # persistent tensors
            bias6 = constp.tile([128, DT], f32, tag="b6")
            va = [persist.tile([128, VA], bf16, tag=f"va{nt}", name=f"va{nt}")
                  for nt in range(NT)]
            aoT = [persist.tile([128, SEQ], bf16, tag=f"ao{j}", name=f"aoT{j}")
                   for j in range(DT)]
            wfcT = [persist.tile([128, DIM], bf16, tag=f"wfcT{j}",
                                 name=f"wfcT{j}") for j in range(DT)]
            xT = [persist.tile([128, SEQ], f32r, tag=f"xT{j}", name=f"xT{j}")
                  for j in range(DT)]
            wvT = [persist.tile([128, DIM], f32r, tag=f"wvT{j}",
                                name=f"wvT{j}") for j in range(DT)]

            # ---- persistent loads: everything is pre-transposed in DRAM.
            # Split so only the ~3.9MB needed by pair-0's qkv matmuls sits
            # ahead of them in the DMA queues (preamble is bandwidth-bound).
            def load_x():
                for j in range(DT):
                    nc.sync.dma_start(xT[j][:],
                                      xt_d.ap()[j * 128:(j + 1) * 128, :])

            def load_rest():
                for j in range(DT):
                    nc.sync.dma_start(
                        wvT[j][:],
                        wqkvt_d.ap()[j * 128:(j + 1) * 128,
                                     12 * 128:18 * 128])
                for j in range(DT):
                    nc.sync.dma_start(wfcT[j][:],
                                      wfct_d.ap()[j * 128:(j + 1) * 128, :])
                nc.sync.dma_start(bias6[:], bias6_d.ap())

            # ---- v matmuls as 16 self-contained parts (one seq-chunk
            # half each), slotted into the early pairs' chunk streams ----
            def v_parts():
                def vp(nt, h2):
                    lo, hi = (0, 512) if h2 == 0 else (512, 768)
                    def go():
                        psv = psp.tile([128, hi - lo], f32, tag="mm", bufs=2,
                                       name="psv")
                        for j in range(DT):
                            nc.tensor.matmul(psv[:],
                                             xT[j][:, nt * 128:(nt + 1) * 128],
                                             wvT[j][:, lo:hi],
                                             start=(j == 0),
                                             stop=(j == DT - 1))
                        va3 = va[nt][:].rearrange("p (h c) -> p h c", c=DH + 1)
                        nc.vector.tensor_copy(
                            va3[:, lo // DH:hi // DH, 0:DH],
                            psv[:].rearrange("p (h c) -> p h c", c=DH))
                        if h2 == 1:
                            nc.gpsimd.memset(va3[:, :, DH:DH + 1], 1.0)
                    return go
                return [vp(nt, h2) for nt in range(NT) for h2 in range(2)]

            def wt_qkv_parts(p, tiles, split_load=False):
                """Pair p's q_T/k_T: w chunks stream in pre-transposed via
                DMA; 4 matmul parts produce qt/kt in `tiles`."""
                wq_t = [work.tile([128, 256], f32r, tag=f"wq{j}", bufs=2,
                                  name=f"wq{j}_{p}") for j in range(DT)]

                def load():
                    for j in range(DT):
                        for ci, et in enumerate((p, 6 + p)):
                            nc.sync.dma_start(
                                wq_t[j][:, ci * 128:(ci + 1) * 128],
                                wqkvt_d.ap()[j * 128:(j + 1) * 128,
                                             et * 128:(et + 1) * 128])

                def qkmm(ci, half, h2):
                    ps = psp.tile([128, 512], f32, tag="mm", bufs=2,
                                  name="ps")
                    for j in range(DT):
                        nc.tensor.matmul(
                            ps[:],
                            wq_t[j][:, ci * 128:(ci + 1) * 128],
                            xT[j][:, h2 * 512:(h2 + 1) * 512],
                            start=(j == 0), stop=(j == DT - 1))
                    if h2 == 0:
                        t = work.tile([128, SEQ], f32r,
                                      tag=f"qk_{half}{p % 3}", bufs=1,
                                      name=f"qk{half}{p}")
                        tiles[half] = t
                    nc.vector.tensor_copy(
                        tiles[half][:, h2 * 512:(h2 + 1) * 512], ps[:])

                mm_parts = [
                    lambda: qkmm(0, "q", 0), lambda: qkmm(0, "q", 1),
                    lambda: qkmm(1, "k", 0), lambda: qkmm(1, "k", 1)]
                if split_load:
                    return load, mm_parts
                return [load] + mm_parts

            def run_parts(parts):
                for f in parts:
                    f()

            def normalize(p, xi, st):
                """recip of denominator row via DRAM-bounce, broadcast, mul."""
                dsc2 = dscp.tile([1, SEQ], f32, tag="dsc2", bufs=2,
                                 name="dsc2")
                dsc1 = dscp.tile([1, SEQ], f32, tag="dsc1", bufs=2,
                                 name="dsc1")
                nc.sync.dma_start(dsc1[:], st[DH:DH + 1, :])
                den8 = work.tile([128, 8], f32, tag="den8", bufs=2,
                                 name="den8")
                nc.sync.dma_start(
                    den8[:], dsc1[:].rearrange("a (p c) -> (a p) c", c=8))
                recip8 = work.tile([128, 8], f32, tag="recip8", bufs=2,
                                   name="recip8")
                nc.vector.reciprocal(recip8[:], den8[:])
                nc.sync.dma_start(
                    dsc2[:].rearrange("a (p c) -> (a p) c", c=8), recip8[:])
                bc_sb = work.tile([64, SEQ], f32, tag="bc", bufs=2,
                                  name="bc_sb")
                nc.sync.dma_start(bc_sb[:], dsc2[:].broadcast_to([64, SEQ]))
                nc.vector.tensor_mul(
                    aoT[p][xi * 64:(xi + 1) * 64, :], st[0:DH, :], bc_sb[:])

            def drain_po(p, xi, po):
                """Stage [65, SEQ] out of PSUM in one copy, then normalize.
                Copy runs on DVE, keeping ScalarE free for the exp stream."""
                st = work.tile([DH + 1, SEQ], f32, tag="stage", bufs=2,
                               name="st")
                nc.vector.tensor_copy(st[:], po[:])
                normalize(p, xi, st)

            def pair_step(p, qk, PT_prev, bg=()):
                """S(p) chunk-interleaved with PV(p-1) + background parts;
                returns PT(p)."""
                qt, kt = qk["q"], qk["k"]
                L = len(bg)
                PT = {}
                po = {}
                if PT_prev is not None:
                    for xi in range(2):
                        po[xi] = psp.tile([DH + 1, SEQ], f32, tag=f"o{xi}",
                                          bufs=1, name=f"po{xi}")
                for c in range(NT):
                    # bg parts run first: their PSUM allocations then reuse
                    # buffers whose exp (c-1) has already drained, instead of
                    # stalling the PE behind an in-flight exp.
                    for i in range(L * c // NT, L * (c + 1) // NT):
                        bg[i]()
                    if PT_prev is not None:
                        for xi in range(2):
                            hX = 2 * (p - 1) + xi
                            va_h = va[c][:, hX * (DH + 1):(hX + 1) * (DH + 1)]
                            for h2 in range(2):
                                nc.tensor.matmul(
                                    po[xi][:, h2 * 512:(h2 + 1) * 512],
                                    va_h,
                                    PT_prev[(xi, c)][:, h2 * 512:
                                                     (h2 + 1) * 512],
                                    start=(c == 0), stop=(c == NT - 1))
                    # All 4 S matmuls first (the two xi target different
                    # 64-row PE tiles, so B's mms overlap A's), exps after —
                    # no ACT op ever sits between PE instructions.
                    ps_s = {}
                    for xi in range(2):
                        ro = xi * 64
                        ps = psp.tile([128, SEQ], f32, tag="mm", bufs=2,
                                      name="ps_s")
                        for h2 in range(2):
                            nc.tensor.matmul(
                                ps[:, h2 * 512:(h2 + 1) * 512],
                                kt[ro:ro + 64, c * 128:(c + 1) * 128],
                                qt[ro:ro + 64, h2 * 512:(h2 + 1) * 512],
                                start=True, stop=True)
                        ps_s[xi] = ps
                    for xi in range(2):
                        pt = work.tile([128, SEQ], bf16, tag=f"pt{xi}_{c}",
                                       bufs=1, name="pt")
                        nc.scalar.activation(pt[:], ps_s[xi][:], EXP)
                        PT[(xi, c)] = pt
                if PT_prev is not None:
                    for xi in range(2):
                        drain_po(p - 1, xi, po[xi])
                return PT

            def pv_only(p, PT_prev):
                for xi in range(2):
                    po = psp.tile([DH + 1, SEQ], f32, tag=f"o{xi}", bufs=1,
                                  name=f"po{xi}")
                    for c in range(NT):
                        hX = 2 * p + xi
                        va_h = va[c][:, hX * (DH + 1):(hX + 1) * (DH + 1)]
                        for h2 in range(2):
                            nc.tensor.matmul(
                                po[:, h2 * 512:(h2 + 1) * 512],
                                va_h,
                                PT_prev[(xi, c)][:, h2 * 512:(h2 + 1) * 512],
                                start=(c == 0), stop=(c == NT - 1))
                    drain_po(p, xi, po)

            # ---- preamble: pure DMA, then straight into pair 0 ----
            qk_tiles = {p: {} for p in range(6)}
            w0_load, w0_parts = wt_qkv_parts(0, qk_tiles[0], split_load=True)
            load_x()
            w0_load()
            load_rest()
            run_parts(w0_parts)
            w1 = wt_qkv_parts(1, qk_tiles[1])
            vps = v_parts()

            def merge(a, b):
                out, ia, ib = [], 0, 0
                while ia < len(a) or ib < len(b):
                    if ia * len(b) <= ib * len(a) and ia < len(a):
                        out.append(a[ia]); ia += 1
                    elif ib < len(b):
                        out.append(b[ib]); ib += 1
                    else:
                        out.append(a[ia]); ia += 1
                return out

            bg_sched = {
                0: merge(w1, vps[:12]),
                1: vps[12:] + wt_qkv_parts(2, qk_tiles[2]),
                2: wt_qkv_parts(3, qk_tiles[3]),
                3: wt_qkv_parts(4, qk_tiles[4]),
                4: wt_qkv_parts(5, qk_tiles[5]),
                5: [],
            }
            PT_cur = None
            for p in range(6):
                PT_cur = pair_step(p, qk_tiles[p], PT_cur, bg_sched[p])
            pv_only(5, PT_cur)

            # ---- fc + bias, transposed output layout: out^T[m,q] with
            # wfc^T chunks stationary and aoT moving. j=0..4 accumulation
            # (fc_a) prefills during the pair-5 drain chains (it needs only
            # aoT[0..4]); the j=5 pair + bias + store (fc_b) follows once
            # aoT[5] is normalized. ----
            pots = {}

            def fc_a(m, tag):
                pot = psp.tile([128, SEQ], f32, tag=tag,
                               bufs=(2 if tag == "mm" else 1),
                               name=f"pot{m}")
                pots[m] = pot
                for j in range(DT - 1):
                    for h2 in range(2):
                        nc.tensor.matmul(
                            pot[:, h2 * 512:(h2 + 1) * 512],
                            wfcT[j][:, m * 128:(m + 1) * 128],
                            aoT[j][:, h2 * 512:(h2 + 1) * 512],
                            start=(j == 0), stop=False)

            def fc_b(m):
                pot = pots[m]
                j = DT - 1
                for h2 in range(2):
                    nc.tensor.matmul(
                        pot[:, h2 * 512:(h2 + 1) * 512],
                        wfcT[j][:, m * 128:(m + 1) * 128],
                        aoT[j][:, h2 * 512:(h2 + 1) * 512],
                        start=False, stop=True)
                yT = work.tile([128, SEQ], f32, tag="y_sb", bufs=2,
                               name="yT")
                nc.vector.tensor_scalar_add(yT[:], pot[:],
                                            bias6[:, m:m + 1])
                nc.sync.dma_start(out_d.ap()[m * 128:(m + 1) * 128, :],
                                  yT[:])

            fc_a(0, "mm")
            fc_a(1, "mm")
            fc_a(2, "o0")
            fc_a(3, "o1")
            fc_b(0)
            fc_a(4, "mm")
            fc_b(1)
            fc_a(5, "mm")
            fc_b(2)
            fc_b(3)
            fc_b(4)
            fc_b(5)

    nc.compile()
    return nc


_NC = None
LAST_RESULTS = None  # BassKernelResults of the most recent run (for profiling)


def kernel(**inputs) -> np.ndarray:
    global _NC, LAST_RESULTS
    x = np.asarray(inputs["x"], dtype=np.float32)
    w_qkv = np.asarray(inputs["w_qkv"], dtype=np.float32)
    w_fc = np.asarray(inputs["w_fc"], dtype=np.float32)
    b_fc = np.asarray(inputs["b_fc"], dtype=np.float32).reshape(1, DIM)

    # Host-side shard/layout prep: pre-transpose operands so the NeuronCore
    # never runs a PE transpose.
    wqkv_t = np.ascontiguousarray(w_qkv.T)                      # [768, 2304]
    wfc_t = np.ascontiguousarray(w_fc.T).astype(ml_dtypes.bfloat16)
    bias6 = np.ascontiguousarray(b_fc.reshape(DT, 128).T)       # [128, 6]

    if _NC is None:
        _NC = build()
    nc = _NC

    in_maps = [
        {"x_t": np.ascontiguousarray(x[b].T), "wqkv_t": wqkv_t,
         "wfc_t": wfc_t, "bias6": bias6}
        for b in range(8)
    ]
    res = run_bass_kernel_spmd(nc, in_maps, core_ids=list(range(8)))
    LAST_RESULTS = res
    # kernel emits out^T [dim, seq]; un-transpose on the host
    out = np.stack([r["out"].T for r in res.results], axis=0)
    return np.ascontiguousarray(out).astype(np.float32)


if __name__ == "__main__":
    rng = np.random.default_rng(0)
    ins = {
        "x": rng.standard_normal((8, SEQ, DIM), dtype=np.float32),
        "w_qkv": (rng.standard_normal((E, DIM), dtype=np.float32) * DIM ** -0.5),
        "w_fc": (rng.standard_normal((DIM, DIM), dtype=np.float32) * DIM ** -0.5),
        "b_fc": (rng.standard_normal((DIM,), dtype=np.float32) * 0.02),
    }
    out = kernel(**ins)
    print("out", out.shape, out.dtype)
